# revision 16
# baseline (speedup 1.0000x reference)
"""Based linear attention (2nd-order Taylor feature map), 8-core Trainium2 kernel.

Sharding: batch x heads. Core c handles batch c//4, heads 3*(c%4)..3*(c%4)+2.
Each core: projections for its heads, causal linear attention via chunked
state form (intra-chunk uses the kernel identity qf.kf = phi(q.k) with
phi(z) = 1 + z/4 + z^2/32 = (z/sqrt(32) + 4/sqrt(32))^2 + 0.5), partial
output projection over its 384 columns of Wo, then a 4-core ReduceScatter
sums partials and leaves each core a 512-row slice of the final output.
"""

import numpy as np

B, T, HID = 2, 2048, 1536
H, DK, DV = 12, 16, 128
HPC = 3              # heads per core
NCORE = 8
C = 128              # time chunk
NCH = T // C         # 16
TS = T // 4          # 512: per-core output slice
DQK = HPC * DK       # 48
DVC = HPC * DV       # 384
VW = DV + 1          # 129: v plus a ones column (denominator rides along)
KT = HID // 128      # 12 contraction tiles
X2 = DK * DK         # 256 second-order features
DF = 1 + DK + X2     # 273
PHI_S = 0.17677669529663687   # 1/sqrt(32)
PHI_B = 0.7071067811865476    # 4/sqrt(32)

_CACHE = None


def _build_module():
    from contextlib import ExitStack

    import concourse.tile as tile
    import concourse.mybir as mybir
    from concourse import bacc
    from concourse.alu_op_type import AluOpType

    fp32 = mybir.dt.float32
    bf16 = mybir.dt.bfloat16
    AF = mybir.ActivationFunctionType

    nc = bacc.Bacc(trn_type="TRN2", target_bir_lowering=False)

    hs = nc.declare_dram_parameter("hs", [T, HID], fp32, isOutput=False)
    wqk = nc.declare_dram_parameter("wqk", [2 * DQK, HID], fp32, isOutput=False)
    wv = nc.declare_dram_parameter("wv", [DVC, HID], fp32, isOutput=False)
    wo = nc.declare_dram_parameter("wo", [HID, DVC], fp32, isOutput=False)
    mask = nc.declare_dram_parameter("mask", [C, C], fp32, isOutput=False)
    sel = nc.declare_dram_parameter("sel", [DK, 3 * 128], fp32, isOutput=False)
    out = nc.declare_dram_parameter("out", [TS, HID], fp32, isOutput=True)

    opart = nc.dram_tensor("opart", [T, HID], bf16)
    rsout = nc.dram_tensor("rsout", [TS, HID], bf16)

    with ExitStack() as ctx:
        tc = ctx.enter_context(tile.TileContext(nc, num_cores=NCORE))

        pers = ctx.enter_context(tc.tile_pool(name="pers", bufs=1))
        hstp_cm = tc.tile_pool(name="hstp", bufs=1)
        hstp = hstp_cm.__enter__()
        stg_cm = tc.tile_pool(name="stg", bufs=2)
        stg = stg_cm.__enter__()

        hsT = [hstp.tile([128, T], bf16, name=f"hsT{i}", tag=f"hsT{i}") for i in range(KT)]
        wqkT = pers.tile([128, KT * 2 * DQK], bf16, name="wqkT", tag="wqkT")
        wvkT = [pers.tile([128, DVC + DQK], bf16, name=f"wvkT{i}", tag=f"wvkT{i}") for i in range(KT)]
        woT = [pers.tile([128, HID], bf16, name=f"woT{j}", tag=f"woT{j}") for j in range(HPC)]
        qkT = pers.tile([2 * DQK, T], bf16, name="qkT", tag="qkT")
        v_sb = [pers.tile([128, HPC * VW + DQK], bf16, name=f"v{i}", tag=f"v{i}") for i in range(NCH)]
        y3T = [pers.tile([128, T], bf16, name=f"y3T{h}", tag=f"y3T{h}") for h in range(HPC)]
        mask_sb = pers.tile([C, C], fp32, name="mask_sb", tag="mask")
        sel_sb = pers.tile([DK, 3 * 128], bf16, name="sel_sb", tag="sel")
        nc.gpsimd.dma_start(out=sel_sb[:], in_=sel[:])
        qh = [pers.tile([DK, T], bf16, name=f"qh{h}", tag=f"qh{h}") for h in range(HPC)]
        kh = [pers.tile([DK, T], bf16, name=f"kh{h}", tag=f"kh{h}") for h in range(HPC)]
        Q0 = [pers.tile([1 + DK, T], bf16, name=f"Q0_{h}", tag=f"Q0_{h}") for h in range(HPC)]
        Qx0 = [pers.tile([128, T], bf16, name=f"Qx0_{h}", tag=f"Qx0_{h}") for h in range(HPC)]
        Qx1 = [pers.tile([128, T], bf16, name=f"Qx1_{h}", tag=f"Qx1_{h}") for h in range(HPC)]
        phib = pers.tile([128, 1], fp32, name="phib", tag="phib")
        nc.gpsimd.memset(phib[:], PHI_B)

        nc.sync.dma_start(out=mask_sb[:], in_=mask[:])

        # ---- weight prep: cast-load (SWDGE casts f32->bf16) + DMA transposes
        wq_sb = stg.tile([DQK, HID], bf16, tag="wq")
        wk_sb = stg.tile([DQK, HID], bf16, tag="wk")
        nc.gpsimd.dma_start(out=wq_sb[:], in_=wqk[0:DQK, :])
        nc.gpsimd.dma_start(out=wk_sb[:], in_=wqk[DQK : 2 * DQK, :])
        for i in range(KT):
            nc.sync.dma_start(
                out=wqkT[:, i * 2 * DQK : i * 2 * DQK + DQK],
                in_=wq_sb[:, i * 128 : (i + 1) * 128],
                transpose=True,
            )
            nc.sync.dma_start(
                out=wqkT[:, i * 2 * DQK + DQK : (i + 1) * 2 * DQK],
                in_=wk_sb[:, i * 128 : (i + 1) * 128],
                transpose=True,
            )
            nc.sync.dma_start(
                out=wvkT[i][:, DVC : DVC + DQK],
                in_=wk_sb[:, i * 128 : (i + 1) * 128],
                transpose=True,
            )
        for s in range(3):
            wv_sb = stg.tile([128, HID], bf16, tag="wv")
            nc.gpsimd.dma_start(out=wv_sb[:], in_=wv[s * 128 : (s + 1) * 128, :])
            for i in range(KT):
                nc.sync.dma_start(
                    out=wvkT[i][:, s * 128 : (s + 1) * 128],
                    in_=wv_sb[:, i * 128 : (i + 1) * 128],
                    transpose=True,
                )
        for s in range(KT):
            wo_sb = stg.tile([128, DVC], bf16, tag="wo")
            nc.gpsimd.dma_start(out=wo_sb[:], in_=wo[s * 128 : (s + 1) * 128, :])
            for j in range(HPC):
                nc.sync.dma_start(
                    out=woT[j][:, s * 128 : (s + 1) * 128],
                    in_=wo_sb[:, j * 128 : (j + 1) * 128],
                    transpose=True,
                )

        # ---- hidden states: cast-load + transpose to hsT [c, t]
        for t in range(NCH):
            hs_sb = stg.tile([128, HID], bf16, tag="hs")
            nc.gpsimd.dma_start(out=hs_sb[:], in_=hs[t * 128 : (t + 1) * 128, :])
            for i in range(KT):
                nc.sync.dma_start(
                    out=hsT[i][:, t * 128 : (t + 1) * 128],
                    in_=hs_sb[:, i * 128 : (i + 1) * 128],
                    transpose=True,
                )

        # ---- q/k projection: qkT [96, T] (rows 0:48 = qT heads, 48:96 = kT)
        pq_cm = tc.tile_pool(name="pq", bufs=2, space="PSUM")
        pq = pq_cm.__enter__()
        for tch in range(T // 512):
            ps = pq.tile([2 * DQK, 512], fp32, tag="qk", padded_shape=[2 * DQK, 512])
            for i in range(KT):
                nc.tensor.matmul(
                    ps[:],
                    lhsT=wqkT[:, i * 2 * DQK : (i + 1) * 2 * DQK],
                    rhs=hsT[i][:, tch * 512 : (tch + 1) * 512],
                    start=(i == 0),
                    stop=(i == KT - 1),
                )
            nc.vector.tensor_copy(out=qkT[:, tch * 512 : (tch + 1) * 512], in_=ps[:])

        # ---- v (+k replica) projection: per t-tile [128, 384+48]
        for t in range(NCH):
            ps = pq.tile([128, DVC + DQK], fp32, tag="v", padded_shape=[128, 512])
            for i in range(KT):
                nc.tensor.matmul(
                    ps[:],
                    lhsT=hsT[i][:, t * 128 : (t + 1) * 128],
                    rhs=wvkT[i][:],
                    start=(i == 0),
                    stop=(i == KT - 1),
                )
            for h in range(HPC):
                nc.vector.tensor_copy(
                    out=v_sb[t][:, h * VW : h * VW + DV],
                    in_=ps[:, h * DV : (h + 1) * DV],
                )
                nc.gpsimd.memset(v_sb[t][:, h * VW + DV : h * VW + DV + 1], 1.0)
            nc.scalar.copy(
                out=v_sb[t][:, HPC * VW : HPC * VW + DQK], in_=ps[:, DVC : DVC + DQK]
            )

        stg_cm.__exit__(None, None, None)
        hstp_cm.__exit__(None, None, None)
        pq_cm.__exit__(None, None, None)

        # ---- transposed query features via PE selection matmuls
        # Qx0[p, t] = q_{p//16} * q_{p%16};  Qx1: rows 8..15 of the i index
        with tc.tile_pool(name="feat", bufs=2, space="PSUM") as feat, \
                tc.tile_pool(name="featsb", bufs=2) as featsb:
            for h in range(HPC):
                nc.sync.dma_start(out=qh[h][:], in_=qkT[h * DK : (h + 1) * DK, :])
                nc.sync.dma_start(
                    out=kh[h][:], in_=qkT[DQK + h * DK : DQK + (h + 1) * DK, :]
                )
                nc.gpsimd.memset(Q0[h][0:1, :], 1.0)
                nc.sync.dma_start(out=Q0[h][1 : 1 + DK, :], in_=qkT[h * DK : (h + 1) * DK, :])
                for tch in range(T // 512):
                    tsl = slice(tch * 512, (tch + 1) * 512)
                    psA = feat.tile([128, 512], fp32, tag="psA", padded_shape=[128, 512])
                    psB1 = feat.tile([128, 512], fp32, tag="psB1", padded_shape=[128, 512])
                    psB2 = feat.tile([128, 512], fp32, tag="psB2", padded_shape=[128, 512])
                    nc.tensor.matmul(
                        psA[:], lhsT=sel_sb[:, 0:128], rhs=qh[h][:, tsl],
                        start=True, stop=True,
                    )
                    nc.tensor.matmul(
                        psB1[:], lhsT=sel_sb[:, 128:256], rhs=qh[h][:, tsl],
                        start=True, stop=True,
                    )
                    nc.tensor.matmul(
                        psB2[:], lhsT=sel_sb[:, 256:384], rhs=qh[h][:, tsl],
                        start=True, stop=True,
                    )
                    sbA = featsb.tile([128, 512], bf16, tag="sbA")
                    nc.vector.tensor_copy(out=sbA[:], in_=psA[:])
                    nc.vector.tensor_tensor(
                        out=Qx0[h][:, tsl], in0=sbA[:], in1=psB1[:], op=AluOpType.mult
                    )
                    nc.vector.tensor_tensor(
                        out=Qx1[h][:, tsl], in0=sbA[:], in1=psB2[:], op=AluOpType.mult
                    )

        # ---- attention per head
        att_cm = ExitStack()
        ah = att_cm.enter_context(tc.tile_pool(name="ah", bufs=1))
        chp = att_cm.enter_context(tc.tile_pool(name="chp", bufs=3))
        pst = att_cm.enter_context(tc.tile_pool(name="pst", bufs=1, space="PSUM"))
        pnum = att_cm.enter_context(tc.tile_pool(name="pnum", bufs=4, space="PSUM"))
        pkv = att_cm.enter_context(tc.tile_pool(name="pkv", bufs=1, space="PSUM"))
        for h in range(HPC):
            kv_f32 = [
                ah.tile([1 + DK, VW], fp32, name="kvf0", tag="kvf0"),
                ah.tile([128, VW], fp32, name="kvf1", tag="kvf1"),
                ah.tile([128, VW], fp32, name="kvf2", tag="kvf2"),
            ]
            kv_sb = [
                ah.tile([1 + DK, VW], bf16, name="kvs0", tag="kvs0"),
                ah.tile([128, VW], bf16, name="kvs1", tag="kvs1"),
                ah.tile([128, VW], bf16, name="kvs2", tag="kvs2"),
            ]
            for b_ in range(3):
                nc.gpsimd.memset(kv_f32[b_][:], 0.0)

            for i in range(NCH):
                sl = slice(i * C, (i + 1) * C)
                vset = v_sb[i][:, h * VW : (h + 1) * VW]

                # intra-chunk: Z[s,tq] = k_s . q_tq, then S = phi(Z) * mask
                st_ps = pst.tile([C, C], fp32, tag="st", padded_shape=[C, 512])
                nc.tensor.matmul(
                    st_ps[:], lhsT=kh[h][:, sl], rhs=qh[h][:, sl], start=True, stop=True
                )
                u = chp.tile([C, C], fp32, tag="u")
                nc.scalar.activation(
                    out=u[:], in_=st_ps[:], func=AF.Square, bias=phib[:], scale=PHI_S
                )
                stm = chp.tile([C, C], bf16, tag="stm")
                nc.vector.scalar_tensor_tensor(
                    out=stm[:],
                    in0=u[:],
                    scalar=0.5,
                    in1=mask_sb[:],
                    op0=AluOpType.add,
                    op1=AluOpType.mult,
                )

                # numerator (+ denominator in col 128)
                num_ps = pnum.tile([C, VW], fp32, tag="num", padded_shape=[C, 512])
                nc.tensor.matmul(
                    num_ps[:], lhsT=stm[:], rhs=vset, start=True, stop=(i == 0)
                )
                if i > 0:
                    nc.tensor.matmul(
                        num_ps[:], lhsT=Q0[h][:, sl], rhs=kv_sb[0][:],
                        start=False, stop=False,
                    )
                    nc.tensor.matmul(
                        num_ps[:], lhsT=Qx0[h][:, sl], rhs=kv_sb[1][:],
                        start=False, stop=False,
                    )
                    nc.tensor.matmul(
                        num_ps[:], lhsT=Qx1[h][:, sl], rhs=kv_sb[2][:],
                        start=False, stop=True,
                    )

                # state update with this chunk's keys (unused after last chunk)
                if i < NCH - 1:
                    kf = chp.tile([C, DF], bf16, tag="kf")
                    nc.gpsimd.memset(kf[:, 0:1], 1.0)
                    k3s = v_sb[i][:, HPC * VW + h * DK : HPC * VW + (h + 1) * DK]
                    nc.vector.tensor_scalar_mul(kf[:, 1 : 1 + DK], k3s, 0.25)
                    nc.vector.scalar_tensor_tensor(
                        out=kf[:, 1 + DK : DF].rearrange("p (i j) -> p i j", i=DK),
                        in0=k3s.unsqueeze(1).broadcast_to((C, DK, DK)),
                        scalar=0.03125,
                        in1=k3s.unsqueeze(2).broadcast_to((C, DK, DK)),
                        op0=AluOpType.mult,
                        op1=AluOpType.mult,
                    )
                    kv_ps = [
                        pkv.tile([1 + DK, VW], fp32, name=f"kvp0_{h}_{i}", tag="kvp0", padded_shape=[1 + DK, 512]),
                        pkv.tile([128, VW], fp32, name=f"kvp1_{h}_{i}", tag="kvp1", padded_shape=[128, 512]),
                        pkv.tile([128, VW], fp32, name=f"kvp2_{h}_{i}", tag="kvp2", padded_shape=[128, 512]),
                    ]
                    nc.tensor.matmul(
                        kv_ps[0][:], lhsT=kf[:, 0 : 1 + DK], rhs=vset,
                        start=True, stop=True,
                    )
                    nc.tensor.matmul(
                        kv_ps[1][:], lhsT=kf[:, 1 + DK : 1 + DK + 128], rhs=vset,
                        start=True, stop=True,
                    )
                    nc.tensor.matmul(
                        kv_ps[2][:], lhsT=kf[:, 1 + DK + 128 : DF], rhs=vset,
                        start=True, stop=True,
                    )
                    for b_ in range(3):
                        nc.vector.tensor_tensor(
                            out=kv_f32[b_][:], in0=kv_f32[b_][:], in1=kv_ps[b_][:],
                            op=AluOpType.add,
                        )
                        nc.scalar.copy(out=kv_sb[b_][:], in_=kv_f32[b_][:])

                # y = num / den, then transpose into y3T[h]
                r = chp.tile([C, 1], fp32, tag="r")
                nc.vector.reciprocal(out=r[:], in_=num_ps[:, DV : DV + 1])
                ysb = chp.tile([C, DV], bf16, tag="y")
                nc.vector.tensor_scalar_mul(ysb[:], num_ps[:, 0:DV], r[:])
                nc.sync.dma_start(out=y3T[h][:, sl], in_=ysb[:], transpose=True)

        att_cm.close()

        # ---- partial output projection: opart[t,:] = y3.T @ Wo3T
        po = ctx.enter_context(tc.tile_pool(name="po", bufs=3, space="PSUM"))
        ost = ctx.enter_context(tc.tile_pool(name="ost", bufs=3))
        for t in range(NCH):
            osb = ost.tile([128, HID], bf16, tag="osb")
            for ns in range(HID // 512):
                ps = po.tile([128, 512], fp32, tag="o", padded_shape=[128, 512])
                for kb in range(HPC):
                    nc.tensor.matmul(
                        ps[:],
                        lhsT=y3T[kb][:, t * 128 : (t + 1) * 128],
                        rhs=woT[kb][:, ns * 512 : (ns + 1) * 512],
                        start=(kb == 0),
                        stop=(kb == HPC - 1),
                    )
                nc.scalar.copy(out=osb[:, ns * 512 : (ns + 1) * 512], in_=ps[:])
            nc.sync.dma_start(out=opart[t * 128 : (t + 1) * 128, :], in_=osb[:])

        # ---- sum partials across the 4 cores sharing a batch; keep own slice
        nc.gpsimd.collective_compute(
            "ReduceScatter",
            AluOpType.add,
            replica_groups=[[0, 1, 2, 3], [4, 5, 6, 7]],
            ins=[opart[:]],
            outs=[rsout[:]],
        )

        fo = ctx.enter_context(tc.tile_pool(name="fo", bufs=2))
        for t in range(TS // 128):
            fb = fo.tile([128, HID], bf16, tag="fb")
            nc.sync.dma_start(out=fb[:], in_=rsout[t * 128 : (t + 1) * 128, :])
            ff = fo.tile([128, HID], fp32, tag="ff")
            nc.vector.tensor_copy(out=ff[:], in_=fb[:])
            nc.sync.dma_start(out=out[t * 128 : (t + 1) * 128, :], in_=ff[:])

    nc.compile()
    return nc


def _in_maps(inputs):
    hs = np.asarray(inputs["hidden_states"], np.float32)
    Wq = np.asarray(inputs["Wq"], np.float32)
    Wk = np.asarray(inputs["Wk"], np.float32)
    Wv = np.asarray(inputs["Wv"], np.float32)
    Wo = np.asarray(inputs["Wo"], np.float32)
    mask = np.triu(np.ones((C, C), np.float32))
    m = np.arange(128)
    k = np.arange(DK)[:, None]
    selA = (k == (m % DK)[None, :]).astype(np.float32)
    selB1 = (k == (m // DK)[None, :]).astype(np.float32)
    selB2 = (k == (8 + m // DK)[None, :]).astype(np.float32)
    sel = np.concatenate([selA, selB1, selB2], axis=1)
    maps = []
    for c in range(NCORE):
        b, h0 = c // 4, HPC * (c % 4)
        maps.append(
            {
                "hs": np.ascontiguousarray(hs[b]),
                "wqk": np.ascontiguousarray(
                    np.concatenate(
                        [Wq[h0 * DK : (h0 + HPC) * DK], Wk[h0 * DK : (h0 + HPC) * DK]], 0
                    )
                ),
                "wv": np.ascontiguousarray(Wv[h0 * DV : (h0 + HPC) * DV]),
                "wo": np.ascontiguousarray(Wo[:, h0 * DV : (h0 + HPC) * DV]),
                "mask": mask,
                "sel": sel,
            }
        )
    return maps


def _assemble(results):
    out = np.empty((B, T, HID), np.float32)
    for c in range(NCORE):
        b, r = c // 4, c % 4
        out[b, r * TS : (r + 1) * TS, :] = results[c]["out"]
    return out


def kernel(**inputs) -> np.ndarray:
    global _CACHE
    if _CACHE is None:
        _CACHE = _build_module()
    from concourse.bass_utils import run_bass_kernel_spmd

    res = run_bass_kernel_spmd(_CACHE, _in_maps(inputs), list(range(NCORE)))
    return _assemble(res.results)


# revision 17
# speedup vs baseline: 2.0894x; 2.0894x over previous
"""Based linear attention (2nd-order Taylor feature map), 8-core Trainium2 kernel.

Sharding: batch x heads. Core c handles batch c//4, heads 3*(c%4)..3*(c%4)+2.
Each core: projections for its heads, causal linear attention via chunked
state form (intra-chunk uses the kernel identity qf.kf = phi(q.k) with
phi(z) = 1 + z/4 + z^2/32 = (z/sqrt(32) + 4/sqrt(32))^2 + 0.5), partial
output projection over its 384 columns of Wo, then a 4-core ReduceScatter
sums partials and leaves each core a 512-row slice of the final output.
"""

import numpy as np

B, T, HID = 2, 2048, 1536
H, DK, DV = 12, 16, 128
HPC = 3              # heads per core
NCORE = 8
C = 128              # time chunk
NCH = T // C         # 16
TS = T // 4          # 512: per-core output slice
DQK = HPC * DK       # 48
DVC = HPC * DV       # 384
VW = DV + 1          # 129: v plus a ones column (denominator rides along)
KT = HID // 128      # 12 contraction tiles
X2 = DK * DK         # 256 second-order features
DF = 1 + DK + X2     # 273
PHI_S = 0.17677669529663687   # 1/sqrt(32)
PHI_B = 0.7071067811865476    # 4/sqrt(32)

_CACHE = None


def _build_module():
    from contextlib import ExitStack

    import concourse.tile as tile
    import concourse.mybir as mybir
    from concourse import bacc
    from concourse.alu_op_type import AluOpType

    fp32 = mybir.dt.float32
    bf16 = mybir.dt.bfloat16
    AF = mybir.ActivationFunctionType

    nc = bacc.Bacc(trn_type="TRN2", target_bir_lowering=False)

    hs = nc.declare_dram_parameter("hs", [T, HID], fp32, isOutput=False)
    wqk = nc.declare_dram_parameter("wqk", [2 * DQK, HID], fp32, isOutput=False)
    wv = nc.declare_dram_parameter("wv", [DVC, HID], fp32, isOutput=False)
    wo = nc.declare_dram_parameter("wo", [HID, DVC], fp32, isOutput=False)
    mask = nc.declare_dram_parameter("mask", [C, C], fp32, isOutput=False)
    sel = nc.declare_dram_parameter("sel", [DK, 3 * 128], fp32, isOutput=False)
    iden = nc.declare_dram_parameter("iden", [128, 128], fp32, isOutput=False)
    out = nc.declare_dram_parameter("out", [TS, HID], fp32, isOutput=True)

    opart = nc.dram_tensor("opart", [T, HID], bf16)
    rsout = nc.dram_tensor("rsout", [TS, HID], bf16)

    with ExitStack() as ctx:
        tc = ctx.enter_context(tile.TileContext(nc, num_cores=NCORE))

        pers = ctx.enter_context(tc.tile_pool(name="pers", bufs=1))
        hstp_cm = tc.tile_pool(name="hstp", bufs=1)
        hstp = hstp_cm.__enter__()
        stg_cm = tc.tile_pool(name="stg", bufs=2)
        stg = stg_cm.__enter__()

        hsT = [hstp.tile([128, T], bf16, name=f"hsT{i}", tag=f"hsT{i}") for i in range(KT)]
        wqkT = pers.tile([128, KT * 2 * DQK], bf16, name="wqkT", tag="wqkT")
        wvkT = [pers.tile([128, DVC + DQK], bf16, name=f"wvkT{i}", tag=f"wvkT{i}") for i in range(KT)]
        woT = [pers.tile([128, HID], bf16, name=f"woT{j}", tag=f"woT{j}") for j in range(HPC)]
        qkT = pers.tile([2 * DQK, T], bf16, name="qkT", tag="qkT")
        v_sb = [pers.tile([128, HPC * VW + DQK], bf16, name=f"v{i}", tag=f"v{i}") for i in range(NCH)]
        y3T = [pers.tile([128, T], bf16, name=f"y3T{h}", tag=f"y3T{h}") for h in range(HPC)]
        mask_sb = pers.tile([C, C], fp32, name="mask_sb", tag="mask")
        sel_sb = pers.tile([DK, 3 * 128], bf16, name="sel_sb", tag="sel")
        nc.gpsimd.dma_start(out=sel_sb[:], in_=sel[:])
        iden_sb = pers.tile([128, 128], bf16, name="iden_sb", tag="iden")
        nc.gpsimd.dma_start(out=iden_sb[:], in_=iden[:])

        evac_ctr = [0]

        def pe_T(ptp, dst, src, kdim):
            tp = ptp.tile(
                [128, kdim], bf16, name=f"tp{evac_ctr[0]}", tag="tp",
                padded_shape=[128, 1024],
            )
            nc.tensor.transpose(tp[:], src, iden_sb[0:kdim, 0:kdim])
            if evac_ctr[0] % 2 == 0:
                nc.vector.tensor_copy(out=dst, in_=tp[:])
            else:
                nc.scalar.copy(out=dst, in_=tp[:])
            evac_ctr[0] += 1
        qh = [pers.tile([DK, T], bf16, name=f"qh{h}", tag=f"qh{h}") for h in range(HPC)]
        kh = [pers.tile([DK, T], bf16, name=f"kh{h}", tag=f"kh{h}") for h in range(HPC)]
        Q0 = [pers.tile([1 + DK, T], bf16, name=f"Q0_{h}", tag=f"Q0_{h}") for h in range(HPC)]
        Qx0 = [pers.tile([128, T], bf16, name=f"Qx0_{h}", tag=f"Qx0_{h}") for h in range(HPC)]
        Qx1 = [pers.tile([128, T], bf16, name=f"Qx1_{h}", tag=f"Qx1_{h}") for h in range(HPC)]
        phib = pers.tile([128, 1], fp32, name="phib", tag="phib")
        nc.gpsimd.memset(phib[:], PHI_B)

        nc.sync.dma_start(out=mask_sb[:], in_=mask[:])

        # ---- weight prep: cast-load (SWDGE casts f32->bf16) + PE transposes
        ptp_cm = tc.tile_pool(name="ptp", bufs=3, space="PSUM")
        ptp = ptp_cm.__enter__()
        wq_sb = stg.tile([DQK, HID], bf16, tag="wq")
        wk_sb = stg.tile([DQK, HID], bf16, tag="wk")
        nc.gpsimd.dma_start(out=wq_sb[:], in_=wqk[0:DQK, :])
        nc.gpsimd.dma_start(out=wk_sb[:], in_=wqk[DQK : 2 * DQK, :])
        for i in range(KT):
            pe_T(ptp, wqkT[:, i * 2 * DQK : i * 2 * DQK + DQK],
                 wq_sb[:, i * 128 : (i + 1) * 128], DQK)
            pe_T(ptp, wqkT[:, i * 2 * DQK + DQK : (i + 1) * 2 * DQK],
                 wk_sb[:, i * 128 : (i + 1) * 128], DQK)
            pe_T(ptp, wvkT[i][:, DVC : DVC + DQK],
                 wk_sb[:, i * 128 : (i + 1) * 128], DQK)
        for s in range(3):
            wv_sb = stg.tile([128, HID], bf16, tag="wv")
            nc.gpsimd.dma_start(out=wv_sb[:], in_=wv[s * 128 : (s + 1) * 128, :])
            for i in range(KT):
                pe_T(ptp, wvkT[i][:, s * 128 : (s + 1) * 128],
                     wv_sb[:, i * 128 : (i + 1) * 128], 128)
        for s in range(KT):
            wo_sb = stg.tile([128, DVC], bf16, tag="wo")
            nc.gpsimd.dma_start(out=wo_sb[:], in_=wo[s * 128 : (s + 1) * 128, :])
            for j in range(HPC):
                pe_T(ptp, woT[j][:, s * 128 : (s + 1) * 128],
                     wo_sb[:, j * 128 : (j + 1) * 128], 128)

        # ---- hidden states: cast-load + transpose to hsT [c, t]
        for t in range(NCH):
            hs_sb = stg.tile([128, HID], bf16, tag="hs")
            nc.gpsimd.dma_start(out=hs_sb[:], in_=hs[t * 128 : (t + 1) * 128, :])
            for i in range(KT):
                pe_T(ptp, hsT[i][:, t * 128 : (t + 1) * 128],
                     hs_sb[:, i * 128 : (i + 1) * 128], 128)

        ptp_cm.__exit__(None, None, None)

        # ---- q/k projection: qkT [96, T] (rows 0:48 = qT heads, 48:96 = kT)
        pq_cm = tc.tile_pool(name="pq", bufs=2, space="PSUM")
        pq = pq_cm.__enter__()
        for tch in range(T // 512):
            ps = pq.tile([2 * DQK, 512], fp32, tag="qk", padded_shape=[2 * DQK, 512])
            for i in range(KT):
                nc.tensor.matmul(
                    ps[:],
                    lhsT=wqkT[:, i * 2 * DQK : (i + 1) * 2 * DQK],
                    rhs=hsT[i][:, tch * 512 : (tch + 1) * 512],
                    start=(i == 0),
                    stop=(i == KT - 1),
                )
            nc.vector.tensor_copy(out=qkT[:, tch * 512 : (tch + 1) * 512], in_=ps[:])

        # ---- v (+k replica) projection: per t-tile [128, 384+48]
        for t in range(NCH):
            ps = pq.tile([128, DVC + DQK], fp32, tag="v", padded_shape=[128, 512])
            for i in range(KT):
                nc.tensor.matmul(
                    ps[:],
                    lhsT=hsT[i][:, t * 128 : (t + 1) * 128],
                    rhs=wvkT[i][:],
                    start=(i == 0),
                    stop=(i == KT - 1),
                )
            for h in range(HPC):
                nc.vector.tensor_copy(
                    out=v_sb[t][:, h * VW : h * VW + DV],
                    in_=ps[:, h * DV : (h + 1) * DV],
                )
                nc.gpsimd.memset(v_sb[t][:, h * VW + DV : h * VW + DV + 1], 1.0)
            nc.scalar.copy(
                out=v_sb[t][:, HPC * VW : HPC * VW + DQK], in_=ps[:, DVC : DVC + DQK]
            )

        # ---- transposed query features via PE selection matmuls
        # Qx0[p, t] = q_{p//16} * q_{p%16};  Qx1: rows 8..15 of the i index
        with tc.tile_pool(name="feat", bufs=1, space="PSUM") as feat, \
                tc.tile_pool(name="featsb", bufs=2) as featsb:
            for h in range(HPC):
                nc.sync.dma_start(out=qh[h][:], in_=qkT[h * DK : (h + 1) * DK, :])
                nc.sync.dma_start(
                    out=kh[h][:], in_=qkT[DQK + h * DK : DQK + (h + 1) * DK, :]
                )
                nc.gpsimd.memset(Q0[h][0:1, :], 1.0)
                nc.sync.dma_start(out=Q0[h][1 : 1 + DK, :], in_=qkT[h * DK : (h + 1) * DK, :])
                for tch in range(T // 512):
                    tsl = slice(tch * 512, (tch + 1) * 512)
                    psA = feat.tile([128, 512], fp32, tag="psA", padded_shape=[128, 512])
                    psB1 = feat.tile([128, 512], fp32, tag="psB1", padded_shape=[128, 512])
                    psB2 = feat.tile([128, 512], fp32, tag="psB2", padded_shape=[128, 512])
                    nc.tensor.matmul(
                        psA[:], lhsT=sel_sb[:, 0:128], rhs=qh[h][:, tsl],
                        start=True, stop=True,
                    )
                    nc.tensor.matmul(
                        psB1[:], lhsT=sel_sb[:, 128:256], rhs=qh[h][:, tsl],
                        start=True, stop=True,
                    )
                    nc.tensor.matmul(
                        psB2[:], lhsT=sel_sb[:, 256:384], rhs=qh[h][:, tsl],
                        start=True, stop=True,
                    )
                    sbA = featsb.tile([128, 512], bf16, tag="sbA")
                    nc.vector.tensor_copy(out=sbA[:], in_=psA[:])
                    nc.vector.tensor_tensor(
                        out=Qx0[h][:, tsl], in0=sbA[:], in1=psB1[:], op=AluOpType.mult
                    )
                    nc.vector.tensor_tensor(
                        out=Qx1[h][:, tsl], in0=sbA[:], in1=psB2[:], op=AluOpType.mult
                    )

        stg_cm.__exit__(None, None, None)
        hstp_cm.__exit__(None, None, None)
        pq_cm.__exit__(None, None, None)

        # ---- attention per head
        att_cm = ExitStack()
        ah = att_cm.enter_context(tc.tile_pool(name="ah", bufs=1))
        chp = att_cm.enter_context(tc.tile_pool(name="chp", bufs=3))
        pst = att_cm.enter_context(tc.tile_pool(name="pst", bufs=2, space="PSUM"))
        pnum = att_cm.enter_context(tc.tile_pool(name="pnum", bufs=2, space="PSUM"))
        pkv = att_cm.enter_context(tc.tile_pool(name="pkv", bufs=1, space="PSUM"))
        ytp = att_cm.enter_context(tc.tile_pool(name="ytp", bufs=1, space="PSUM"))
        for h in range(HPC):
            kv_ps = [
                pkv.tile([1 + DK, VW], fp32, name=f"kvp0_{h}", tag="kvp0", padded_shape=[1 + DK, 512]),
                pkv.tile([128, VW], fp32, name=f"kvp1_{h}", tag="kvp1", padded_shape=[128, 512]),
                pkv.tile([128, VW], fp32, name=f"kvp2_{h}", tag="kvp2", padded_shape=[128, 512]),
            ]
            kv_sb = [
                ah.tile([1 + DK, VW], bf16, name="kvs0", tag="kvs0"),
                ah.tile([128, VW], bf16, name="kvs1", tag="kvs1"),
                ah.tile([128, VW], bf16, name="kvs2", tag="kvs2"),
            ]

            for i in range(NCH):
                sl = slice(i * C, (i + 1) * C)
                vset = v_sb[i][:, h * VW : (h + 1) * VW]

                # intra-chunk: Z[s,tq] = k_s . q_tq, then S = phi(Z) * mask
                st_ps = pst.tile([C, C], fp32, tag="st", padded_shape=[C, 512])
                nc.tensor.matmul(
                    st_ps[:], lhsT=kh[h][:, sl], rhs=qh[h][:, sl], start=True, stop=True
                )
                u = chp.tile([C, C], fp32, tag="u")
                nc.scalar.activation(
                    out=u[:], in_=st_ps[:], func=AF.Square, bias=phib[:], scale=PHI_S
                )
                stm = chp.tile([C, C], bf16, tag="stm")
                nc.vector.scalar_tensor_tensor(
                    out=stm[:],
                    in0=u[:],
                    scalar=0.5,
                    in1=mask_sb[:],
                    op0=AluOpType.add,
                    op1=AluOpType.mult,
                )

                # numerator (+ denominator in col 128)
                num_ps = pnum.tile([C, VW], fp32, tag="num", padded_shape=[C, 512])
                nc.tensor.matmul(
                    num_ps[:], lhsT=stm[:], rhs=vset, start=True, stop=(i == 0)
                )
                if i > 0:
                    nc.tensor.matmul(
                        num_ps[:], lhsT=Q0[h][:, sl], rhs=kv_sb[0][:],
                        start=False, stop=False,
                    )
                    nc.tensor.matmul(
                        num_ps[:], lhsT=Qx0[h][:, sl], rhs=kv_sb[1][:],
                        start=False, stop=False,
                    )
                    nc.tensor.matmul(
                        num_ps[:], lhsT=Qx1[h][:, sl], rhs=kv_sb[2][:],
                        start=False, stop=True,
                    )

                # state update with this chunk's keys (unused after last chunk)
                if i < NCH - 1:
                    kf = chp.tile([C, DF], bf16, tag="kf")
                    nc.gpsimd.memset(kf[:, 0:1], 1.0)
                    k3s = v_sb[i][:, HPC * VW + h * DK : HPC * VW + (h + 1) * DK]
                    nc.vector.tensor_scalar_mul(kf[:, 1 : 1 + DK], k3s, 0.25)
                    nc.vector.scalar_tensor_tensor(
                        out=kf[:, 1 + DK : DF].rearrange("p (i j) -> p i j", i=DK),
                        in0=k3s.unsqueeze(1).broadcast_to((C, DK, DK)),
                        scalar=0.03125,
                        in1=k3s.unsqueeze(2).broadcast_to((C, DK, DK)),
                        op0=AluOpType.mult,
                        op1=AluOpType.mult,
                    )
                    sgc = i > 0  # accumulate into the open PSUM state
                    nc.tensor.matmul(
                        kv_ps[0][:], lhsT=kf[:, 0 : 1 + DK], rhs=vset,
                        start=(i == 0), stop=(i == 0), skip_group_check=sgc,
                    )
                    nc.tensor.matmul(
                        kv_ps[1][:], lhsT=kf[:, 1 + DK : 1 + DK + 128], rhs=vset,
                        start=(i == 0), stop=(i == 0), skip_group_check=sgc,
                    )
                    nc.tensor.matmul(
                        kv_ps[2][:], lhsT=kf[:, 1 + DK + 128 : DF], rhs=vset,
                        start=(i == 0), stop=(i == 0), skip_group_check=sgc,
                    )
                    for b_ in range(3):
                        nc.scalar.copy(out=kv_sb[b_][:], in_=kv_ps[b_][:])

                # y = num / den, then transpose into y3T[h]
                r = chp.tile([C, 1], fp32, tag="r")
                nc.vector.reciprocal(out=r[:], in_=num_ps[:, DV : DV + 1])
                ysb = chp.tile([C, DV], bf16, tag="y")
                nc.vector.tensor_scalar_mul(ysb[:], num_ps[:, 0:DV], r[:])
                ytp_t = ytp.tile([128, C], bf16, tag="yt", padded_shape=[128, 1024])
                nc.tensor.transpose(ytp_t[:], ysb[:], iden_sb[:])
                nc.vector.tensor_copy(out=y3T[h][:, sl], in_=ytp_t[:])

        att_cm.close()

        # ---- partial output projection: opart[t,:] = y3.T @ Wo3T
        po = ctx.enter_context(tc.tile_pool(name="po", bufs=3, space="PSUM"))
        ost = ctx.enter_context(tc.tile_pool(name="ost", bufs=3))
        for t in range(NCH):
            osb = ost.tile([128, HID], bf16, tag="osb")
            for ns in range(HID // 512):
                ps = po.tile([128, 512], fp32, tag="o", padded_shape=[128, 512])
                for kb in range(HPC):
                    nc.tensor.matmul(
                        ps[:],
                        lhsT=y3T[kb][:, t * 128 : (t + 1) * 128],
                        rhs=woT[kb][:, ns * 512 : (ns + 1) * 512],
                        start=(kb == 0),
                        stop=(kb == HPC - 1),
                    )
                nc.scalar.copy(out=osb[:, ns * 512 : (ns + 1) * 512], in_=ps[:])
            nc.sync.dma_start(out=opart[t * 128 : (t + 1) * 128, :], in_=osb[:])

        # ---- sum partials across the 4 cores sharing a batch; keep own slice
        nc.gpsimd.collective_compute(
            "ReduceScatter",
            AluOpType.add,
            replica_groups=[[0, 1, 2, 3], [4, 5, 6, 7]],
            ins=[opart[:]],
            outs=[rsout[:]],
        )

        fo = ctx.enter_context(tc.tile_pool(name="fo", bufs=2))
        for t in range(TS // 128):
            ff = fo.tile([128, HID], fp32, tag="ff")
            nc.gpsimd.dma_start(out=ff[:], in_=rsout[t * 128 : (t + 1) * 128, :])
            nc.sync.dma_start(out=out[t * 128 : (t + 1) * 128, :], in_=ff[:])

    nc.compile()
    return nc


def _in_maps(inputs):
    hs = np.asarray(inputs["hidden_states"], np.float32)
    Wq = np.asarray(inputs["Wq"], np.float32)
    Wk = np.asarray(inputs["Wk"], np.float32)
    Wv = np.asarray(inputs["Wv"], np.float32)
    Wo = np.asarray(inputs["Wo"], np.float32)
    mask = np.triu(np.ones((C, C), np.float32))
    m = np.arange(128)
    k = np.arange(DK)[:, None]
    selA = (k == (m % DK)[None, :]).astype(np.float32)
    selB1 = (k == (m // DK)[None, :]).astype(np.float32)
    selB2 = (k == (8 + m // DK)[None, :]).astype(np.float32)
    sel = np.concatenate([selA, selB1, selB2], axis=1)
    maps = []
    for c in range(NCORE):
        b, h0 = c // 4, HPC * (c % 4)
        maps.append(
            {
                "hs": np.ascontiguousarray(hs[b]),
                "wqk": np.ascontiguousarray(
                    np.concatenate(
                        [Wq[h0 * DK : (h0 + HPC) * DK], Wk[h0 * DK : (h0 + HPC) * DK]], 0
                    )
                ),
                "wv": np.ascontiguousarray(Wv[h0 * DV : (h0 + HPC) * DV]),
                "wo": np.ascontiguousarray(Wo[:, h0 * DV : (h0 + HPC) * DV]),
                "mask": mask,
                "sel": sel,
                "iden": np.eye(128, dtype=np.float32),
            }
        )
    return maps


def _assemble(results):
    out = np.empty((B, T, HID), np.float32)
    for c in range(NCORE):
        b, r = c // 4, c % 4
        out[b, r * TS : (r + 1) * TS, :] = results[c]["out"]
    return out


def kernel(**inputs) -> np.ndarray:
    global _CACHE
    if _CACHE is None:
        _CACHE = _build_module()
    from concourse.bass_utils import run_bass_kernel_spmd

    res = run_bass_kernel_spmd(_CACHE, _in_maps(inputs), list(range(NCORE)))
    return _assemble(res.results)


# revision 22
# speedup vs baseline: 2.1432x; 1.0257x over previous
"""Based linear attention (2nd-order Taylor feature map), 8-core Trainium2 kernel.

Sharding: batch x heads. Core c handles batch c//4, heads 3*(c%4)..3*(c%4)+2.
Each core: projections for its heads, causal linear attention via chunked
state form (intra-chunk uses the kernel identity qf.kf = phi(q.k) with
phi(z) = 1 + z/4 + z^2/32 = (z/sqrt(32) + 4/sqrt(32))^2 + 0.5), partial
output projection over its 384 columns of Wo, then a 4-core ReduceScatter
sums partials and leaves each core a 512-row slice of the final output.
"""

import numpy as np

B, T, HID = 2, 2048, 1536
H, DK, DV = 12, 16, 128
HPC = 3              # heads per core
NCORE = 8
C = 128              # time chunk
NCH = T // C         # 16
TS = T // 4          # 512: per-core output slice
DQK = HPC * DK       # 48
DVC = HPC * DV       # 384
VW = DV + 1          # 129: v plus a ones column (denominator rides along)
KT = HID // 128      # 12 contraction tiles
X2 = DK * DK         # 256 second-order features
DF = 1 + DK + X2     # 273
PHI_S = 0.17677669529663687   # 1/sqrt(32)
PHI_B = 0.7071067811865476    # 4/sqrt(32)

_CACHE = None


def _build_module():
    from contextlib import ExitStack

    import concourse.tile as tile
    import concourse.mybir as mybir
    from concourse import bacc
    from concourse.alu_op_type import AluOpType

    fp32 = mybir.dt.float32
    bf16 = mybir.dt.bfloat16
    AF = mybir.ActivationFunctionType

    nc = bacc.Bacc(trn_type="TRN2", target_bir_lowering=False, num_swdge_queues=4)

    hs = nc.declare_dram_parameter("hs", [T, HID], fp32, isOutput=False)
    wqk = nc.declare_dram_parameter("wqk", [2 * DQK, HID], fp32, isOutput=False)
    wv = nc.declare_dram_parameter("wv", [DVC, HID], fp32, isOutput=False)
    wo = nc.declare_dram_parameter("wo", [HID, DVC], fp32, isOutput=False)
    mask = nc.declare_dram_parameter("mask", [C, C], fp32, isOutput=False)
    sel = nc.declare_dram_parameter("sel", [DK, 3 * 128], fp32, isOutput=False)
    iden = nc.declare_dram_parameter("iden", [128, 128], fp32, isOutput=False)
    out = nc.declare_dram_parameter("out", [TS, HID], fp32, isOutput=True)

    opart = [nc.dram_tensor(f"opart{j}", [T, 512], bf16) for j in range(HID // 512)]
    rsout = [nc.dram_tensor(f"rsout{j}", [TS, 512], bf16) for j in range(HID // 512)]

    with ExitStack() as ctx:
        tc = ctx.enter_context(tile.TileContext(nc, num_cores=NCORE))

        pers = ctx.enter_context(tc.tile_pool(name="pers", bufs=1))
        hstp_cm = tc.tile_pool(name="hstp", bufs=1)
        hstp = hstp_cm.__enter__()
        stg_cm = tc.tile_pool(name="stg", bufs=2)
        stg = stg_cm.__enter__()

        hsT = [hstp.tile([128, T], bf16, name=f"hsT{i}", tag=f"hsT{i}") for i in range(KT)]
        wqkT = pers.tile([128, KT * 2 * DQK], bf16, name="wqkT", tag="wqkT")
        wvkT = [pers.tile([128, DVC + DQK], bf16, name=f"wvkT{i}", tag=f"wvkT{i}") for i in range(KT)]
        woT = [pers.tile([128, HID], bf16, name=f"woT{j}", tag=f"woT{j}") for j in range(HPC)]
        qkT = pers.tile([2 * DQK, T], bf16, name="qkT", tag="qkT")
        v_sb = [pers.tile([128, HPC * VW + DQK], bf16, name=f"v{i}", tag=f"v{i}") for i in range(NCH)]
        y3T = [pers.tile([128, T], bf16, name=f"y3T{h}", tag=f"y3T{h}") for h in range(HPC)]
        mask_sb = pers.tile([C, C], fp32, name="mask_sb", tag="mask")
        sel_sb = pers.tile([DK, 3 * 128], bf16, name="sel_sb", tag="sel")
        nc.gpsimd.dma_start(out=sel_sb[:], in_=sel[:])
        iden_sb = pers.tile([128, 128], bf16, name="iden_sb", tag="iden")
        nc.gpsimd.dma_start(out=iden_sb[:], in_=iden[:])

        evac_ctr = [0]

        def pe_T(ptp, dst, src, kdim):
            tp = ptp.tile(
                [128, kdim], bf16, name=f"tp{evac_ctr[0]}", tag="tp",
                padded_shape=[128, 1024],
            )
            nc.tensor.transpose(tp[:], src, iden_sb[0:kdim, 0:kdim])
            if evac_ctr[0] % 2 == 0:
                nc.vector.tensor_copy(out=dst, in_=tp[:])
            else:
                nc.scalar.copy(out=dst, in_=tp[:])
            evac_ctr[0] += 1
        qh = [pers.tile([DK, T], bf16, name=f"qh{h}", tag=f"qh{h}") for h in range(HPC)]
        kh = [pers.tile([DK, T], bf16, name=f"kh{h}", tag=f"kh{h}") for h in range(HPC)]
        Q0 = [pers.tile([1 + DK, T], bf16, name=f"Q0_{h}", tag=f"Q0_{h}") for h in range(HPC)]
        Qx0 = [pers.tile([128, T], bf16, name=f"Qx0_{h}", tag=f"Qx0_{h}") for h in range(HPC)]
        Qx1 = [pers.tile([128, T], bf16, name=f"Qx1_{h}", tag=f"Qx1_{h}") for h in range(HPC)]
        phib = pers.tile([128, 1], fp32, name="phib", tag="phib")
        nc.gpsimd.memset(phib[:], PHI_B)

        nc.sync.dma_start(out=mask_sb[:], in_=mask[:])

        # ---- weight prep: cast-load (SWDGE casts f32->bf16) + PE transposes
        ptp_cm = tc.tile_pool(name="ptp", bufs=3, space="PSUM")
        ptp = ptp_cm.__enter__()
        wq_sb = stg.tile([DQK, HID], bf16, tag="wq")
        wk_sb = stg.tile([DQK, HID], bf16, tag="wk")
        nc.gpsimd.dma_start(out=wq_sb[:], in_=wqk[0:DQK, :])
        nc.gpsimd.dma_start(out=wk_sb[:], in_=wqk[DQK : 2 * DQK, :])
        for i in range(KT):
            pe_T(ptp, wqkT[:, i * 2 * DQK : i * 2 * DQK + DQK],
                 wq_sb[:, i * 128 : (i + 1) * 128], DQK)
            pe_T(ptp, wqkT[:, i * 2 * DQK + DQK : (i + 1) * 2 * DQK],
                 wk_sb[:, i * 128 : (i + 1) * 128], DQK)
            pe_T(ptp, wvkT[i][:, DVC : DVC + DQK],
                 wk_sb[:, i * 128 : (i + 1) * 128], DQK)
        for s in range(3):
            wv_sb = stg.tile([128, HID], bf16, tag="wv")
            nc.gpsimd.dma_start(out=wv_sb[:], in_=wv[s * 128 : (s + 1) * 128, :])
            for i in range(KT):
                pe_T(ptp, wvkT[i][:, s * 128 : (s + 1) * 128],
                     wv_sb[:, i * 128 : (i + 1) * 128], 128)
        for s in range(KT):
            wo_sb = stg.tile([128, DVC], bf16, tag="wo")
            nc.gpsimd.dma_start(out=wo_sb[:], in_=wo[s * 128 : (s + 1) * 128, :])
            for j in range(HPC):
                pe_T(ptp, woT[j][:, s * 128 : (s + 1) * 128],
                     wo_sb[:, j * 128 : (j + 1) * 128], 128)

        # ---- hidden states: cast-load + transpose to hsT [c, t]
        for t in range(NCH):
            hs_sb = stg.tile([128, HID], bf16, tag="hs")
            nc.gpsimd.dma_start(out=hs_sb[:], in_=hs[t * 128 : (t + 1) * 128, :])
            for i in range(KT):
                pe_T(ptp, hsT[i][:, t * 128 : (t + 1) * 128],
                     hs_sb[:, i * 128 : (i + 1) * 128], 128)

        ptp_cm.__exit__(None, None, None)

        # ---- q/k projection: qkT [96, T] (rows 0:48 = qT heads, 48:96 = kT)
        pq_cm = tc.tile_pool(name="pq", bufs=2, space="PSUM")
        pq = pq_cm.__enter__()
        for tch in range(T // 512):
            ps = pq.tile([2 * DQK, 512], fp32, tag="qk", padded_shape=[2 * DQK, 512])
            for i in range(KT):
                nc.tensor.matmul(
                    ps[:],
                    lhsT=wqkT[:, i * 2 * DQK : (i + 1) * 2 * DQK],
                    rhs=hsT[i][:, tch * 512 : (tch + 1) * 512],
                    start=(i == 0),
                    stop=(i == KT - 1),
                )
            nc.vector.tensor_copy(out=qkT[:, tch * 512 : (tch + 1) * 512], in_=ps[:])

        # ---- v (+k replica) projection: per t-tile [128, 384+48]
        for t in range(NCH):
            ps = pq.tile([128, DVC + DQK], fp32, tag="v", padded_shape=[128, 512])
            for i in range(KT):
                nc.tensor.matmul(
                    ps[:],
                    lhsT=hsT[i][:, t * 128 : (t + 1) * 128],
                    rhs=wvkT[i][:],
                    start=(i == 0),
                    stop=(i == KT - 1),
                )
            for h in range(HPC):
                nc.vector.tensor_copy(
                    out=v_sb[t][:, h * VW : h * VW + DV],
                    in_=ps[:, h * DV : (h + 1) * DV],
                )
                nc.gpsimd.memset(v_sb[t][:, h * VW + DV : h * VW + DV + 1], 1.0)
            nc.scalar.copy(
                out=v_sb[t][:, HPC * VW : HPC * VW + DQK], in_=ps[:, DVC : DVC + DQK]
            )

        # ---- transposed query features via PE selection matmuls
        # Qx0[p, t] = q_{p//16} * q_{p%16};  Qx1: rows 8..15 of the i index
        with tc.tile_pool(name="feat", bufs=1, space="PSUM") as feat, \
                tc.tile_pool(name="featsb", bufs=2) as featsb:
            for h in range(HPC):
                nc.sync.dma_start(out=qh[h][:], in_=qkT[h * DK : (h + 1) * DK, :])
                nc.sync.dma_start(
                    out=kh[h][:], in_=qkT[DQK + h * DK : DQK + (h + 1) * DK, :]
                )
                nc.gpsimd.memset(Q0[h][0:1, :], 1.0)
                nc.sync.dma_start(out=Q0[h][1 : 1 + DK, :], in_=qkT[h * DK : (h + 1) * DK, :])
                for tch in range(T // 512):
                    tsl = slice(tch * 512, (tch + 1) * 512)
                    psA = feat.tile([128, 512], fp32, tag="psA", padded_shape=[128, 512])
                    psB1 = feat.tile([128, 512], fp32, tag="psB1", padded_shape=[128, 512])
                    psB2 = feat.tile([128, 512], fp32, tag="psB2", padded_shape=[128, 512])
                    nc.tensor.matmul(
                        psA[:], lhsT=sel_sb[:, 0:128], rhs=qh[h][:, tsl],
                        start=True, stop=True,
                    )
                    nc.tensor.matmul(
                        psB1[:], lhsT=sel_sb[:, 128:256], rhs=qh[h][:, tsl],
                        start=True, stop=True,
                    )
                    nc.tensor.matmul(
                        psB2[:], lhsT=sel_sb[:, 256:384], rhs=qh[h][:, tsl],
                        start=True, stop=True,
                    )
                    sbA = featsb.tile([128, 512], bf16, tag="sbA")
                    nc.vector.tensor_copy(out=sbA[:], in_=psA[:])
                    nc.vector.tensor_tensor(
                        out=Qx0[h][:, tsl], in0=sbA[:], in1=psB1[:], op=AluOpType.mult
                    )
                    nc.vector.tensor_tensor(
                        out=Qx1[h][:, tsl], in0=sbA[:], in1=psB2[:], op=AluOpType.mult
                    )

        stg_cm.__exit__(None, None, None)
        hstp_cm.__exit__(None, None, None)
        pq_cm.__exit__(None, None, None)

        # ---- attention per head
        att_cm = ExitStack()
        ah = att_cm.enter_context(tc.tile_pool(name="ah", bufs=1))
        chp = att_cm.enter_context(tc.tile_pool(name="chp", bufs=3))
        pst = att_cm.enter_context(tc.tile_pool(name="pst", bufs=2, space="PSUM"))
        pnum = att_cm.enter_context(tc.tile_pool(name="pnum", bufs=2, space="PSUM"))
        pkv = att_cm.enter_context(tc.tile_pool(name="pkv", bufs=1, space="PSUM"))
        ytp = att_cm.enter_context(tc.tile_pool(name="ytp", bufs=1, space="PSUM"))
        for h in range(HPC):
            kv_ps = [
                pkv.tile([1 + DK, VW], fp32, name=f"kvp0_{h}", tag="kvp0", padded_shape=[1 + DK, 512]),
                pkv.tile([128, VW], fp32, name=f"kvp1_{h}", tag="kvp1", padded_shape=[128, 512]),
                pkv.tile([128, VW], fp32, name=f"kvp2_{h}", tag="kvp2", padded_shape=[128, 512]),
            ]
            kv_sb = [
                ah.tile([1 + DK, VW], bf16, name="kvs0", tag="kvs0"),
                ah.tile([128, VW], bf16, name="kvs1", tag="kvs1"),
                ah.tile([128, VW], bf16, name="kvs2", tag="kvs2"),
            ]

            for i in range(NCH):
                sl = slice(i * C, (i + 1) * C)
                vset = v_sb[i][:, h * VW : (h + 1) * VW]

                # intra-chunk: Z[s,tq] = k_s . q_tq, then S = phi(Z) * mask
                st_ps = pst.tile([C, C], fp32, tag="st", padded_shape=[C, 512])
                nc.tensor.matmul(
                    st_ps[:], lhsT=kh[h][:, sl], rhs=qh[h][:, sl], start=True, stop=True
                )
                u = chp.tile([C, C], fp32, tag="u")
                nc.scalar.activation(
                    out=u[:], in_=st_ps[:], func=AF.Square, bias=phib[:], scale=PHI_S
                )
                stm = chp.tile([C, C], bf16, tag="stm")
                nc.vector.scalar_tensor_tensor(
                    out=stm[:],
                    in0=u[:],
                    scalar=0.5,
                    in1=mask_sb[:],
                    op0=AluOpType.add,
                    op1=AluOpType.mult,
                )

                # numerator (+ denominator in col 128)
                num_ps = pnum.tile([C, VW], fp32, tag="num", padded_shape=[C, 512])
                nc.tensor.matmul(
                    num_ps[:], lhsT=stm[:], rhs=vset, start=True, stop=(i == 0)
                )
                if i > 0:
                    nc.tensor.matmul(
                        num_ps[:], lhsT=Q0[h][:, sl], rhs=kv_sb[0][:],
                        start=False, stop=False,
                    )
                    nc.tensor.matmul(
                        num_ps[:], lhsT=Qx0[h][:, sl], rhs=kv_sb[1][:],
                        start=False, stop=False,
                    )
                    nc.tensor.matmul(
                        num_ps[:], lhsT=Qx1[h][:, sl], rhs=kv_sb[2][:],
                        start=False, stop=True,
                    )

                # state update with this chunk's keys (unused after last chunk)
                if i < NCH - 1:
                    kf = chp.tile([C, DF], bf16, tag="kf")
                    nc.gpsimd.memset(kf[:, 0:1], 1.0)
                    k3s = v_sb[i][:, HPC * VW + h * DK : HPC * VW + (h + 1) * DK]
                    nc.vector.tensor_scalar_mul(kf[:, 1 : 1 + DK], k3s, 0.25)
                    nc.vector.scalar_tensor_tensor(
                        out=kf[:, 1 + DK : DF].rearrange("p (i j) -> p i j", i=DK),
                        in0=k3s.unsqueeze(1).broadcast_to((C, DK, DK)),
                        scalar=0.03125,
                        in1=k3s.unsqueeze(2).broadcast_to((C, DK, DK)),
                        op0=AluOpType.mult,
                        op1=AluOpType.mult,
                    )
                    sgc = i > 0  # accumulate into the open PSUM state
                    nc.tensor.matmul(
                        kv_ps[0][:], lhsT=kf[:, 0 : 1 + DK], rhs=vset,
                        start=(i == 0), stop=(i == 0), skip_group_check=sgc,
                    )
                    nc.tensor.matmul(
                        kv_ps[1][:], lhsT=kf[:, 1 + DK : 1 + DK + 128], rhs=vset,
                        start=(i == 0), stop=(i == 0), skip_group_check=sgc,
                    )
                    nc.tensor.matmul(
                        kv_ps[2][:], lhsT=kf[:, 1 + DK + 128 : DF], rhs=vset,
                        start=(i == 0), stop=(i == 0), skip_group_check=sgc,
                    )
                    for b_ in range(3):
                        nc.scalar.copy(out=kv_sb[b_][:], in_=kv_ps[b_][:])

                # y = num / den, then transpose into y3T[h]
                r = chp.tile([C, 1], fp32, tag="r")
                nc.vector.reciprocal(out=r[:], in_=num_ps[:, DV : DV + 1])
                ysb = chp.tile([C, DV], bf16, tag="y")
                nc.vector.tensor_scalar_mul(ysb[:], num_ps[:, 0:DV], r[:])
                ytp_t = ytp.tile([128, C], bf16, tag="yt", padded_shape=[128, 1024])
                nc.tensor.transpose(ytp_t[:], ysb[:], iden_sb[:])
                nc.vector.tensor_copy(out=y3T[h][:, sl], in_=ytp_t[:])

        att_cm.close()

        # ---- partial output projection: opart[t,:] = y3.T @ Wo3T.
        # Column-slab order (ns outer) so each 512-col slab's ReduceScatter
        # overlaps the next slab's matmuls.
        po = ctx.enter_context(tc.tile_pool(name="po", bufs=3, space="PSUM"))
        ost = ctx.enter_context(tc.tile_pool(name="ost", bufs=4))
        fo = ctx.enter_context(tc.tile_pool(name="fo", bufs=2))
        for ns in range(HID // 512):
            csl = slice(ns * 512, (ns + 1) * 512)
            for t in range(NCH):
                ps = po.tile([128, 512], fp32, tag="o", padded_shape=[128, 512])
                for kb in range(HPC):
                    nc.tensor.matmul(
                        ps[:],
                        lhsT=y3T[kb][:, t * 128 : (t + 1) * 128],
                        rhs=woT[kb][:, csl],
                        start=(kb == 0),
                        stop=(kb == HPC - 1),
                    )
                osb = ost.tile([128, 512], bf16, tag="osb")
                nc.scalar.copy(out=osb[:], in_=ps[:])
                nc.sync.dma_start(out=opart[ns][t * 128 : (t + 1) * 128, :], in_=osb[:])
        for ns in range(HID // 512):
            csl = slice(ns * 512, (ns + 1) * 512)
            nc.gpsimd.collective_compute(
                "ReduceScatter",
                AluOpType.add,
                replica_groups=[[0, 1, 2, 3], [4, 5, 6, 7]],
                ins=[opart[ns][:]],
                outs=[rsout[ns][:]],
            )
            for t in range(TS // 128):
                ff = fo.tile([128, 512], fp32, tag="ff")
                nc.gpsimd.dma_start(out=ff[:], in_=rsout[ns][t * 128 : (t + 1) * 128, :])
                nc.sync.dma_start(out=out[t * 128 : (t + 1) * 128, csl], in_=ff[:])

    nc.compile()
    return nc


def _in_maps(inputs):
    hs = np.asarray(inputs["hidden_states"], np.float32)
    Wq = np.asarray(inputs["Wq"], np.float32)
    Wk = np.asarray(inputs["Wk"], np.float32)
    Wv = np.asarray(inputs["Wv"], np.float32)
    Wo = np.asarray(inputs["Wo"], np.float32)
    mask = np.triu(np.ones((C, C), np.float32))
    m = np.arange(128)
    k = np.arange(DK)[:, None]
    selA = (k == (m % DK)[None, :]).astype(np.float32)
    selB1 = (k == (m // DK)[None, :]).astype(np.float32)
    selB2 = (k == (8 + m // DK)[None, :]).astype(np.float32)
    sel = np.concatenate([selA, selB1, selB2], axis=1)
    maps = []
    for c in range(NCORE):
        b, h0 = c // 4, HPC * (c % 4)
        maps.append(
            {
                "hs": np.ascontiguousarray(hs[b]),
                "wqk": np.ascontiguousarray(
                    np.concatenate(
                        [Wq[h0 * DK : (h0 + HPC) * DK], Wk[h0 * DK : (h0 + HPC) * DK]], 0
                    )
                ),
                "wv": np.ascontiguousarray(Wv[h0 * DV : (h0 + HPC) * DV]),
                "wo": np.ascontiguousarray(Wo[:, h0 * DV : (h0 + HPC) * DV]),
                "mask": mask,
                "sel": sel,
                "iden": np.eye(128, dtype=np.float32),
            }
        )
    return maps


def _assemble(results):
    out = np.empty((B, T, HID), np.float32)
    for c in range(NCORE):
        b, r = c // 4, c % 4
        out[b, r * TS : (r + 1) * TS, :] = results[c]["out"]
    return out


def kernel(**inputs) -> np.ndarray:
    global _CACHE
    if _CACHE is None:
        _CACHE = _build_module()
    from concourse.bass_utils import run_bass_kernel_spmd

    res = run_bass_kernel_spmd(_CACHE, _in_maps(inputs), list(range(NCORE)))
    return _assemble(res.results)
